# revision 42
# baseline (speedup 1.0000x reference)
"""GRU cell kernel for Trainium2, data-parallel over 8 NeuronCores.

Reference computation (B=4096, I=H=2048, C=I+H=4096):
    combined   = [x, h]                                   [B, C]
    to_update  = sigmoid(combined @ W_update.T + b_u)     [B, H]
    to_select  = sigmoid(combined @ W_select.T + b_s)     [B, H]
    updated    = h * to_update
    new_comb   = [x, updated]
    predictions= tanh(new_comb @ W_predict.T + b_p)
    h_new      = h * (1 - to_select) + predictions * to_select
    = h + to_select * (predictions - h)

Sharding: batch split 8 ways (512 rows/core), weights replicated.
On-chip layout is [feature, batch] (transposed); weight tiles are the
stationary matmul operand, activation tiles [128c, 512b] the moving one.

Precision: the update gate runs fully in fp8e4 with DoubleRow perf mode
(2 contraction rows/cycle, ~2x the bf16 matmul rate). The select and
predict gates are error-critical (select's error multiplies (p - h)) and
stay bf16 except for their first K8=4 contraction k-tiles, which also
run fp8 DoubleRow: all weights are pre-scaled by 64 on the host (exact
in bf16, and placing the fp8 slices in e4m3's normal range) so the fp8
and bf16 partial products share one psum scale, and each activation
folds the 1/64 back out via its input scale. Measured rel err 0.0153
(gate 2e-2), bit-identical across runs and matching the numpy
quantization simulation to 0.3%. h is used in bf16 everywhere (no fp32
h upload); psum accumulation is fp32 throughout.

Schedule: dummy matmuls on a memset tile warm the PE clock (HAM) while
the head DMAs land; the fp8 gate goes first because its input bytes are
half (2 MB fp8 combined + 0.5 MB/block weights), minimizing the time to
first full accumulation group. bf16 activations stream in behind the
fp8 weight blocks during the update phase. The last predict tile is
split into four quarter-batch groups so the activation/blend/store
chains of earlier quarters hide under later quarters' matmuls.
"""

from contextlib import ExitStack

import numpy as np
import ml_dtypes

import concourse.bass as bass  # noqa: F401  (kept for parity with docs)
import concourse.tile as tile
import concourse.mybir as mybir
from concourse import bacc
from concourse.bass_utils import run_bass_kernel_spmd

BF16 = mybir.dt.bfloat16
F8 = mybir.dt.float8e4
F32 = mybir.dt.float32
NPBF16 = ml_dtypes.bfloat16
NPF8 = ml_dtypes.float8_e4m3

B, I, H = 4096, 2048, 2048
C = I + H
NCORES = 8
BS = B // NCORES            # 512 batch rows per core
P = 128                     # SBUF partitions
HT = H // P                 # 16 output-row tiles
IT = I // P                 # 16 x feature tiles
CT = C // P                 # 32 contraction tiles
WSCALE = 64.0               # fp8 weight pre-scale (W*64 ~ N(0,1))
K8 = 4                      # leading k-tiles of s/p gates run fp8 DoubleRow
NWARM = 14                  # PE-clock warm-up matmuls
ACT_F = mybir.ActivationFunctionType
DR = mybir.MatmulPerfMode.DoubleRow

_PROGRAM = None


def _build_program():
    nc = bacc.Bacc("TRN2")

    xh8 = nc.dram_tensor("xh8", [P, CT, BS], F8, kind="ExternalInput")
    xhb = nc.dram_tensor("xhb", [P, CT, BS], BF16, kind="ExternalInput")
    Wu8 = nc.dram_tensor("Wu8", [HT, P, CT, P], F8, kind="ExternalInput")
    # s/p weights are pre-scaled x64 on the host (exact in bf16) so their
    # fp8 leading-k slices share one psum scale; ACTs fold 1/64 back out.
    Ws = nc.dram_tensor("Ws", [HT, P, CT, P], BF16, kind="ExternalInput")
    Wp = nc.dram_tensor("Wp", [HT, P, CT, P], BF16, kind="ExternalInput")
    Ws8 = nc.dram_tensor("Ws8", [HT, P, K8, P], F8, kind="ExternalInput")
    Wp8 = nc.dram_tensor("Wp8", [HT, P, K8, P], F8, kind="ExternalInput")
    bias = nc.dram_tensor("bias", [P, 3 * HT], F32, kind="ExternalInput")
    out = nc.dram_tensor("out", [HT, P, BS], F32, kind="ExternalOutput")

    with tile.TileContext(nc) as tc, ExitStack() as ctx:
        singles = ctx.enter_context(tc.tile_pool(name="singles", bufs=1))
        wpool = ctx.enter_context(tc.tile_pool(name="wpool", bufs=4))
        pspool = ctx.enter_context(tc.tile_pool(name="ps", bufs=8, space="PSUM"))
        work = wpool

        # Dummy matmuls on a zeroed tile: keeps the PE HAM at full clock
        # while the first input DMAs are still in flight.
        warm = singles.tile([P, BS], BF16, name="warm")
        nc.vector.memset(warm[:], 0.0)
        for _ in range(NWARM):
            wps = pspool.tile([P, BS], F32, tag="ps", name="wps")
            nc.tensor.matmul(wps, warm[:, 0:P], warm[:], start=True, stop=True)

        bias_sb = singles.tile([P, 3 * HT], F32, name="bias_sb")

        # Activation buffers are split into chunk tiles: tile-framework
        # dependencies are whole-tile, so a single big tile would make the
        # first matmul wait for every chunk DMA instead of just its own.
        C8N, CBN = 2, 4                  # k-tiles per chunk: 32/C8N, 32/CBN
        comb8s = [
            singles.tile([P, CT // C8N, BS], F8, name=f"comb8_{c}")
            for c in range(C8N)
        ]
        combbs = [
            singles.tile([P, CT // CBN, BS], BF16, name=f"combb_{c}")
            for c in range(CBN)
        ]
        newc = singles.tile([P, HT, BS], BF16, name="newc")
        usb = singles.tile([P, HT, BS], BF16, name="usb")
        sel = singles.tile([P, HT, BS], F32, name="sel")

        def c8ap(k0, k1, b0=0, b1=BS):
            """fp8 combined, k-tiles [k0:k1) — must stay within one chunk"""
            w = CT // C8N
            c, l0 = k0 // w, k0 % w
            return comb8s[c][:, l0:l0 + (k1 - k0), b0:b1]

        def cbap(n, b0, b1):
            """bf16 combined, k-tile n, batch cols [b0:b1)"""
            w = CT // CBN
            return combbs[n // w][:, n % w, b0:b1]

        # Head DMAs: fp8 weight blocks interleaved with the fp8 activation
        # chunks, so update-gate groups 0-2 pipeline at wire rate.
        w8_pre = []
        for j in range(3):
            t = wpool.tile([P, CT, P], F8, tag="w8", name=f"w8_{j}")
            w8_pre.append(t)
        w8c = CT // C8N
        nc.sync.dma_start(w8_pre[0][:], Wu8[0])
        for c in range(C8N):
            nc.sync.dma_start(
                comb8s[c][:], xh8[:, w8c * c:w8c * (c + 1), :]
            )
        nc.sync.dma_start(w8_pre[1][:], Wu8[1])
        # bias is only needed at the first ACT (~4 us after group 0), so its
        # trigger sits behind the critical head transfers.
        nc.sync.dma_start(bias_sb[:], bias[:])
        nc.sync.dma_start(w8_pre[2][:], Wu8[2])

        # ---- update gate, fp8 DoubleRow ----
        for i in range(HT):
            if i < 3:
                w8 = w8_pre[i]
            else:
                w8 = wpool.tile([P, CT, P], F8, tag="w8", name="w8")
                nc.sync.dma_start(w8[:], Wu8[i])
            ps = pspool.tile([P, BS], F32, tag="ps", name="ps")
            for n in range(CT // 2):
                nc.tensor.matmul(
                    ps,
                    w8[:, 2 * n:2 * n + 2, :],
                    c8ap(2 * n, 2 * n + 2),
                    start=(n == 0),
                    stop=(n == CT // 2 - 1),
                    perf_mode=DR,
                )
            nc.scalar.activation(
                usb[:, i:i + 1, :], ps[:], ACT_F.Sigmoid,
                bias=bias_sb[:, i:i + 1], scale=1.0 / WSCALE,
            )
            # bf16 activations stream in behind the fp8 weight blocks
            if i in (2, 4, 6, 8):
                c = i // 2 - 1
                wbc = CT // CBN
                nc.sync.dma_start(
                    combbs[c][:], xhb[:, wbc * c:wbc * (c + 1), :]
                )

        # updated = h * u  (fires when the bf16 h tiles land; feeds predict)
        for i in range(HT):
            nc.vector.tensor_mul(
                newc[:, i:i + 1, :], cbap(HT + i, 0, BS), usb[:, i:i + 1, :]
            )

        def hybrid_group(W8, Wb, i, c0, c1, use_newc):
            """psum group: k-tiles [0:K8) fp8 DoubleRow + [K8:CT) bf16.

            Returns the psum tile holding 64*z over batch cols [c0:c1)."""
            wq = wpool.tile([P, K8, P], F8, tag="wq", name="wq")
            nc.sync.dma_start(wq[:], W8[i])
            wb = wpool.tile([P, CT - K8, P], BF16, tag="wb", name="wb")
            nc.sync.dma_start(wb[:], Wb[i, :, K8:CT, :])
            ps = pspool.tile([P, BS], F32, tag="ps", name="ps")
            for n in range(K8 // 2):
                nc.tensor.matmul(
                    ps[:, 0:c1 - c0],
                    wq[:, 2 * n:2 * n + 2, :],
                    c8ap(2 * n, 2 * n + 2, c0, c1),
                    start=(n == 0), stop=False,
                    perf_mode=DR,
                )
            for n in range(K8, CT):
                rhs = (
                    newc[:, n - HT, c0:c1] if (use_newc and n >= HT)
                    else cbap(n, c0, c1)
                )
                nc.tensor.matmul(
                    ps[:, 0:c1 - c0], wb[:, n - K8:n - K8 + 1, :], rhs,
                    start=False, stop=(n == CT - 1),
                )
            return ps

        # ---- select gate: fp8 head + bf16 tail; sel fp32 for the blend ----
        for i in range(HT):
            ps = hybrid_group(Ws8, Ws, i, 0, BS, use_newc=False)
            nc.scalar.activation(
                sel[:, i:i + 1, :], ps[:], ACT_F.Sigmoid,
                bias=bias_sb[:, HT + i:HT + i + 1], scale=1.0 / WSCALE,
            )

        # ---- predict gate + blend ----
        def pchain(i, c0, c1, ps, chunks=1):
            p_t = work.tile([P, BS], F32, tag="p", name="p_t")
            d = work.tile([P, BS], F32, tag="d", name="d")
            o = work.tile([P, BS], F32, tag="o", name="o")
            cw = (c1 - c0) // chunks
            for q in range(chunks):
                a, b = q * cw, (q + 1) * cw
                ga, gb = c0 + a, c0 + b
                nc.scalar.activation(
                    p_t[:, a:b], ps[:, a:b], ACT_F.Tanh,
                    bias=bias_sb[:, 2 * HT + i:2 * HT + i + 1],
                    scale=1.0 / WSCALE,
                )
                nc.vector.tensor_sub(
                    d[:, a:b], p_t[:, a:b], cbap(HT + i, ga, gb)
                )
                nc.vector.tensor_mul(d[:, a:b], d[:, a:b], sel[:, i, ga:gb])
                nc.vector.tensor_add(
                    o[:, a:b], cbap(HT + i, ga, gb), d[:, a:b]
                )
                nc.sync.dma_start(out[i, :, ga:gb], o[:, a:b])

        for i in range(HT):
            if i < HT - 1:
                ps = hybrid_group(Wp8, Wp, i, 0, BS, use_newc=True)
                pchain(i, 0, BS, ps)
            else:
                # last tile: pure bf16, quartered over batch columns so the
                # blend chains hide under later quarters' matmuls.
                wb = singles.tile([P, CT, P], BF16, name="wbp")
                nc.sync.dma_start(wb[:], Wp[i])
                for q in range(4):
                    c0, c1 = q * BS // 4, (q + 1) * BS // 4
                    ps = pspool.tile([P, BS], F32, tag="ps", name="ps")
                    for n in range(CT):
                        rhs = (
                            cbap(n, c0, c1) if n < HT
                            else newc[:, n - HT, c0:c1]
                        )
                        nc.tensor.matmul(
                            ps[:, 0:c1 - c0], wb[:, n:n + 1, :], rhs,
                            start=(n == 0), stop=(n == CT - 1),
                        )
                    pchain(i, c0, c1, ps)

    nc.finalize()
    return nc


def _get_program():
    global _PROGRAM
    if _PROGRAM is None:
        _PROGRAM = _build_program()
    return _PROGRAM


def _pack_weight(w, scale, npdtype):
    """[H, C] fp32 -> [HT, P, CT, P] with [i, p, n, m] = W[i*128+m, n*128+p].

    Slice [i, :, n, :] is the stationary operand (lhsT = W.T tile) for
    contraction tile n of output tile i.
    """
    wb = (np.asarray(w, dtype=np.float32) * scale).astype(npdtype)
    return np.ascontiguousarray(wb.reshape(HT, P, CT, P).transpose(0, 3, 2, 1))


def _prep_inputs(x, h, W_update, b_update, W_select, b_select, W_predict,
                 b_predict):
    x = np.asarray(x, dtype=np.float32)
    h = np.asarray(h, dtype=np.float32)
    comb = np.concatenate([x, h], axis=1)

    Wu8 = _pack_weight(W_update, WSCALE, NPF8)
    Ws = _pack_weight(W_select, WSCALE, NPBF16)
    Wp = _pack_weight(W_predict, WSCALE, NPBF16)
    Ws8 = np.ascontiguousarray(
        _pack_weight(W_select, WSCALE, NPF8)[:, :, :K8, :]
    )
    Wp8 = np.ascontiguousarray(
        _pack_weight(W_predict, WSCALE, NPF8)[:, :, :K8, :]
    )
    bias = np.ascontiguousarray(
        np.concatenate(
            [
                np.asarray(b_update, dtype=np.float32).reshape(HT, P).T,
                np.asarray(b_select, dtype=np.float32).reshape(HT, P).T,
                np.asarray(b_predict, dtype=np.float32).reshape(HT, P).T,
            ],
            axis=1,
        )
    )

    in_maps = []
    for c in range(NCORES):
        rows = slice(c * BS, (c + 1) * BS)
        ct = np.ascontiguousarray(
            comb[rows].T.reshape(CT, P, BS).transpose(1, 0, 2)
        )
        in_maps.append(
            {
                "xh8": ct.astype(NPF8),
                "xhb": ct.astype(NPBF16),
                "Wu8": Wu8,
                "Ws": Ws,
                "Wp": Wp,
                "Ws8": Ws8,
                "Wp8": Wp8,
                "bias": bias,
            }
        )
    return in_maps


def kernel(x, h, W_update, b_update, W_select, b_select, W_predict, b_predict,
           _trace=False):
    nc = _get_program()
    in_maps = _prep_inputs(
        x, h, W_update, b_update, W_select, b_select, W_predict, b_predict
    )
    res = run_bass_kernel_spmd(
        nc, in_maps, core_ids=list(range(NCORES)), trace=_trace
    )
    h_new = np.empty((B, H), dtype=np.float32)
    for c in range(NCORES):
        rows = slice(c * BS, (c + 1) * BS)
        h_new[rows] = res.results[c]["out"].reshape(H, BS).T
    if _trace:
        return h_new, res
    return h_new
